# revision 15
# baseline (speedup 1.0000x reference)
"""Multi-head attention (B=4, N=2048, DIM=1024, H=16, HD=64) on 8 TRN2 cores.

Sharding: tensor-parallel over heads — 2 heads per core. The reference omits
the output projection, so each core's output is a disjoint 128-column slice of
the final [B, N, 1024]; no collectives are needed.

Per-core device kernel (bf16 compute, fp32 PSUM accumulation):
  - QKV projection from a single pass over x^T: q^T,k^T produced transposed
    [outch, tokens] (weights stationary), v produced natural [tokens, outch]
    (x tiles stationary) with bias folded in as a K=1 accumulation.
  - scores^T = k^T.T @ q^T per (batch, head): K=64 contraction; head A lives
    on partitions 0-63 and head B on 64-127, so the two heads' matmuls
    row-tile the PE array and run concurrently.
  - exp on ScalarE over two-bank [128, 1024] PSUM tiles -> bf16 SBUF.
  - out^T = [v | 1]^T @ expT accumulated over k tiles; row 64 is the softmax
    denominator. Normalization: DVE fast reciprocal of row 64, broadcast to
    64 partitions via a K=1 PE outer product, DVE multiply, DMA out.
"""

import numpy as np
import ml_dtypes

import concourse.bass as bass
import concourse.bacc as bacc
import concourse.mybir as mybir
from concourse.bass_utils import run_bass_kernel_spmd
from concourse.tile import TileContext

B, N, DIM, H = 4, 2048, 1024, 16
HD = DIM // H
SCALE = 1.0 / np.sqrt(HD)
TOK = B * N               # 8192 tokens
NCORES = 8
HPC = H // NCORES         # heads per core = 2

BF16 = mybir.dt.bfloat16
F32 = mybir.dt.float32
AF = mybir.ActivationFunctionType


def _patch_tile_drain():
    """walrus here rejects Drain instructions carrying >2 sem waits; emit the
    waits as standalone single-wait instructions instead."""
    def _split_drain_and_barrier(self, tick_clock, wait_clock):
        nc = self.nc
        clock = tick_clock.global_clock
        sems = wait_clock.sems.allocated()
        nc.sync.drain()
        for proc, tick in enumerate(list(clock)):
            if tick <= 0:
                continue
            handle = sems.get(proc)
            if handle is None:
                continue
            mult = 16 if handle.name.startswith("DMA") else 1
            nc.sync.wait_ge(handle, tick * mult)
        nc.all_engine_barrier()
        assert self.sems is not None
        popped = nc._tile_sem_poison_stack.pop()
        assert popped is self._sem_poison
        nc.clear_and_free_semaphores(list(self.sems.allocated().values()))
        nc.all_engine_barrier()

    TileContext._drain_and_barrier = _split_drain_and_barrier


NT = TOK // 512           # 16 token tiles of 512 for the projection
KT = 8                    # 1024 / 128 contraction tiles
QT = N // 512             # 4 q tiles per (b, h)
KTOK = N // 128           # 16 k-token tiles per (b, h)
VROW = 2 * (HD + 1)       # 130: [vA | 1 | vB | 1] per token tile


def build_graph(debug_dumps=False):
    nc = bacc.Bacc("TRN2", target_bir_lowering=False, debug=False)
    xt = nc.declare_dram_parameter("xt", [DIM, TOK], BF16, isOutput=False)
    wqk = nc.declare_dram_parameter("wqk", [DIM, 2 * HPC * HD], BF16, isOutput=False)
    wv = nc.declare_dram_parameter("wv", [DIM, HPC * HD], BF16, isOutput=False)
    bqk = nc.declare_dram_parameter("bqk", [2 * HPC * HD, 1], F32, isOutput=False)
    bv = nc.declare_dram_parameter("bv", [1, HPC * HD], BF16, isOutput=False)
    out = nc.declare_dram_parameter("out", [HPC, B, HD, N], F32, isOutput=True)
    F32R = mybir.dt.float32r
    NTB = N // 512            # 4 proj token-tiles per batch
    KTOK_B = N // 128         # 16 k-token tiles per batch

    with TileContext(nc) as tc:
        with (
            tc.tile_pool(name="const", bufs=1) as constp,
            tc.tile_pool(name="qk", bufs=1) as qkp,
            tc.tile_pool(name="xin", bufs=2) as xinp,
            tc.tile_pool(name="exps", bufs=28) as expp,
            tc.tile_pool(name="outs", bufs=4) as outp,
            tc.tile_pool(name="rcs", bufs=4) as rcp,
        ):
            # ---- constants ----
            wqk_s = constp.tile([128, KT * 256], BF16)
            for kt in range(KT):
                nc.sync.dma_start(out=wqk_s[:, kt * 256:(kt + 1) * 256],
                                  in_=wqk[kt * 128:(kt + 1) * 128, :])
            wv_s = constp.tile([128, KT * 128], BF16)
            for kt in range(KT):
                nc.sync.dma_start(out=wv_s[:, kt * 128:(kt + 1) * 128],
                                  in_=wv[kt * 128:(kt + 1) * 128, :])
            bqk_s = constp.tile([128, 2], F32)
            for mt in range(2):
                nc.sync.dma_start(out=bqk_s[:, mt:mt + 1],
                                  in_=bqk[mt * 128:(mt + 1) * 128, :])
            bv_s = constp.tile([1, 128], BF16)
            nc.sync.dma_start(out=bv_s[:, :], in_=bv[:, :])
            ones_s = constp.tile([1, 128], BF16)
            nc.vector.memset(ones_s[:, :], 1.0)
            onesk_s = constp.tile([128, 1], BF16)      # K-dim ones for denominators
            nc.vector.memset(onesk_s[:, :], 1.0)
            onesb_s = constp.tile([33, 64], F32)       # bcast lhsT rows 0 / 32
            nc.vector.memset(onesb_s[:, :], 1.0)

            # per-batch activation tensors (lets attention on batch b start
            # as soon as batch b's projection tiles land)
            q_sb = [qkp.tile([128, N], BF16, name=f"q_sb{_b}") for _b in range(B)]
            k_sb = [qkp.tile([128, N], BF16, name=f"k_sb{_b}") for _b in range(B)]
            v_sb = [qkp.tile([128, KTOK_B * 128], BF16, name=f"v_sb{_b}") for _b in range(B)]

            # ---- phase B: QKV projection ----
            with (
                tc.tile_pool(name="qkps", bufs=2, space="PSUM") as qkps,
                tc.tile_pool(name="vps", bufs=2, space="PSUM") as vps,
            ):
                for nt in range(NT):
                    bb, ntb = nt // NTB, nt % NTB
                    xnt = xinp.tile([128, KT * 512], BF16)
                    for kt in range(KT):
                        nc.sync.dma_start(
                            out=xnt[:, kt * 512:(kt + 1) * 512],
                            in_=xt[kt * 128:(kt + 1) * 128, nt * 512:(nt + 1) * 512])
                    for mt in range(2):
                        ps = qkps.tile([128, 512], F32)
                        for kt in range(KT):
                            nc.tensor.matmul(
                                ps[:, :],
                                lhsT=wqk_s[:, kt * 256 + mt * 128: kt * 256 + (mt + 1) * 128],
                                rhs=xnt[:, kt * 512:(kt + 1) * 512],
                                start=(kt == 0), stop=(kt == KT - 1))
                        dst = q_sb[bb] if mt == 0 else k_sb[bb]
                        nc.vector.tensor_scalar_add(
                            dst[:, ntb * 512:(ntb + 1) * 512], ps[:, :],
                            bqk_s[:, mt:mt + 1])
                    for sub in range(4):
                        ttb = ntb * 4 + sub
                        vp = vps.tile([128, 128], F32)
                        for kt in range(KT):
                            nc.tensor.matmul(
                                vp[:, :],
                                lhsT=xnt[:, kt * 512 + sub * 128: kt * 512 + (sub + 1) * 128],
                                rhs=wv_s[:, kt * 128:(kt + 1) * 128],
                                start=(kt == 0), stop=False)
                        nc.tensor.matmul(vp[:, :], lhsT=ones_s[:, :], rhs=bv_s[:, :],
                                         start=False, stop=True)
                        nc.vector.tensor_copy(
                            v_sb[bb][:, ttb * 128:(ttb + 1) * 128], vp[:, :])

            if debug_dumps:
                dq = nc.declare_dram_parameter("dq", [128, N], F32, isOutput=True)
                dqs = outp.tile([128, N], F32, name="dqs", tag="dbg", bufs=1)
                nc.vector.tensor_copy(dqs[:, :], q_sb[0][:, :])
                nc.sync.dma_start(out=dq[:, :], in_=dqs[:, :])

            # ---- phase C: attention ----
            # wave = (b, qt).  Per k-tile: 2 row-tiled score MMs (heads at
            # rows 0-63 / 64-127) into a [128,1024] chunk, exp -> SBUF bf16;
            # then col-tiled av MMs (head A -> psum rows 0-63, head B ->
            # 64-127) and col-paired M=1 denominator MMs; normalize both
            # heads in single [128,512] DVE ops.
            with (
                tc.tile_pool(name="sps", bufs=3, space="PSUM") as sps,
                tc.tile_pool(name="avps", bufs=1, space="PSUM") as avps,
                tc.tile_pool(name="dnps", bufs=1, space="PSUM") as dnps,
            ):
                for b in range(B):
                    for qt in range(QT):
                        qcol = qt * 512
                        echunks = []
                        for kt in range(KTOK_B):
                            kcol = kt * 128
                            s2 = sps.tile([128, 1024], F32, name="s2", tag="s2")
                            for h in range(2):
                                nc.tensor.matmul(
                                    s2[:, h * 512:(h + 1) * 512],
                                    lhsT=k_sb[b][h * 64:(h + 1) * 64, kcol:kcol + 128],
                                    rhs=q_sb[b][h * 64:(h + 1) * 64, qcol:qcol + 512],
                                    start=True, stop=True,
                                    tile_position=(h * 64, 0))
                            e2 = expp.tile([128, 1024], BF16, name="e2", tag="e2")
                            nc.scalar.activation(e2[:, :], s2[:, :], AF.Exp)
                            echunks.append(e2)
                        av = avps.tile([128, 512], F32, name="av")
                        dn2 = dnps.tile([33, 512], F32, name="dn2")
                        for kt in range(KTOK_B):
                            for h in range(2):
                                nc.tensor.matmul(
                                    av[h * 64:(h + 1) * 64, :],
                                    lhsT=v_sb[b][:, kt * 128 + h * 64: kt * 128 + (h + 1) * 64],
                                    rhs=echunks[kt][:, h * 512:(h + 1) * 512],
                                    start=(kt == 0), stop=(kt == KTOK_B - 1),
                                    skip_group_check=True)
                            for h in range(2):
                                nc.tensor.matmul(
                                    dn2[h * 32:h * 32 + 1, :],
                                    lhsT=onesk_s[:, 0:1],
                                    rhs=echunks[kt][:, h * 512:(h + 1) * 512],
                                    start=(kt == 0), stop=(kt == KTOK_B - 1),
                                    skip_group_check=True,
                                    tile_position=(0, h * 32))
                        dn = rcp.tile([33, 512], F32, name="dn", tag="dn")
                        nc.vector.tensor_copy(dn[:, :], dn2[:, :])
                        bc = sps.tile([128, 512], F32, name="bc", tag="s2")
                        for h in range(2):
                            nc.tensor.matmul(
                                bc[h * 64:(h + 1) * 64, :],
                                lhsT=onesb_s[h * 32:h * 32 + 1, 0:64],
                                rhs=dn[h * 32:h * 32 + 1, :],
                                start=True, stop=True)
                        bcs = rcp.tile([128, 512], F32, name="bcs", tag="bcs")
                        nc.vector.tensor_copy(bcs[:, :], bc[:, :])
                        nc.vector.reciprocal_approx_fast(bcs[:, :], bcs[:, :])
                        ot = outp.tile([128, 512], F32)
                        nc.vector.tensor_mul(ot[:, :], av[:, :], bcs[:, :])
                        for h in range(2):
                            nc.sync.dma_start(
                                out=out[h, b, :, qt * 512:(qt + 1) * 512],
                                in_=ot[h * 64:(h + 1) * 64, :])
    nc.compile()
    return nc


_GRAPH = None


def _get_graph():
    global _GRAPH
    if _GRAPH is None:
        _GRAPH = build_graph()
    return _GRAPH


def _make_in_maps(x, w_qkv, b_qkv):
    bf = ml_dtypes.bfloat16
    xt = np.ascontiguousarray(x.reshape(TOK, DIM).T).astype(bf)
    in_maps = []
    for c in range(NCORES):
        hA, hB = HPC * c, HPC * c + 1
        rq = [w_qkv[h * HD:(h + 1) * HD] * SCALE for h in (hA, hB)]
        rk = [w_qkv[DIM + h * HD: DIM + (h + 1) * HD] for h in (hA, hB)]
        rv = [w_qkv[2 * DIM + h * HD: 2 * DIM + (h + 1) * HD] for h in (hA, hB)]
        wqk_c = np.ascontiguousarray(np.concatenate(rq + rk, axis=0).T).astype(bf)
        wv_c = np.ascontiguousarray(np.concatenate(rv, axis=0).T).astype(bf)
        bq = [b_qkv[h * HD:(h + 1) * HD] * SCALE for h in (hA, hB)]
        bk = [b_qkv[DIM + h * HD: DIM + (h + 1) * HD] for h in (hA, hB)]
        bvc = [b_qkv[2 * DIM + h * HD: 2 * DIM + (h + 1) * HD] for h in (hA, hB)]
        bqk_c = np.concatenate(bq + bk).astype(np.float32).reshape(-1, 1)
        bv_c = np.concatenate(bvc).astype(bf).reshape(1, -1)
        in_maps.append({"xt": xt, "wqk": wqk_c, "wv": wv_c,
                        "bqk": np.ascontiguousarray(bqk_c),
                        "bv": np.ascontiguousarray(bv_c)})
    return in_maps


def _run(x, w_qkv, b_qkv, trace=False, tmpdir=None):
    nc = _get_graph()
    in_maps = _make_in_maps(np.asarray(x, dtype=np.float32),
                            np.asarray(w_qkv, dtype=np.float32),
                            np.asarray(b_qkv, dtype=np.float32))
    res = run_bass_kernel_spmd(nc, in_maps, core_ids=list(range(NCORES)),
                               trace=trace, tmpdir=tmpdir)
    full = np.empty((B, N, DIM), dtype=np.float32)
    for c in range(NCORES):
        oc = res.results[c]["out"]          # [HPC, B, HD, N]
        # out[b, q, (HPC*c+hh)*HD + d] = oc[hh, b, d, q]
        full[:, :, c * HPC * HD:(c + 1) * HPC * HD] = \
            oc.transpose(1, 3, 0, 2).reshape(B, N, HPC * HD)
    return full, res


def kernel(x, w_qkv, b_qkv):
    full, _ = _run(x, w_qkv, b_qkv, trace=False)
    return full


# revision 16
# speedup vs baseline: 1.0757x; 1.0757x over previous
"""Multi-head attention (B=4, N=2048, DIM=1024, H=16, HD=64) on 8 TRN2 cores.

Sharding: tensor-parallel over heads — 2 heads per core. The reference omits
the output projection, so each core's output is a disjoint 128-column slice of
the final [B, N, 1024]; no collectives are needed.

Per-core device kernel (bf16 compute, fp32 PSUM accumulation):
  - QKV projection from a single pass over x^T: q^T,k^T produced transposed
    [outch, tokens] (weights stationary), v produced natural [tokens, outch]
    (x tiles stationary) with bias folded in as a K=1 accumulation.
  - scores^T = k^T.T @ q^T per (batch, head): K=64 contraction; head A lives
    on partitions 0-63 and head B on 64-127, so the two heads' matmuls
    row-tile the PE array and run concurrently.
  - exp on ScalarE over two-bank [128, 1024] PSUM tiles -> bf16 SBUF.
  - out^T = [v | 1]^T @ expT accumulated over k tiles; row 64 is the softmax
    denominator. Normalization: DVE fast reciprocal of row 64, broadcast to
    64 partitions via a K=1 PE outer product, DVE multiply, DMA out.
"""

import numpy as np
import ml_dtypes

import concourse.bass as bass
import concourse.bacc as bacc
import concourse.mybir as mybir
from concourse.bass_utils import run_bass_kernel_spmd
from concourse.tile import TileContext

B, N, DIM, H = 4, 2048, 1024, 16
HD = DIM // H
SCALE = 1.0 / np.sqrt(HD)
TOK = B * N               # 8192 tokens
NCORES = 8
HPC = H // NCORES         # heads per core = 2

BF16 = mybir.dt.bfloat16
F32 = mybir.dt.float32
AF = mybir.ActivationFunctionType


def _patch_tile_drain():
    """walrus here rejects Drain instructions carrying >2 sem waits; emit the
    waits as standalone single-wait instructions instead."""
    def _split_drain_and_barrier(self, tick_clock, wait_clock):
        nc = self.nc
        clock = tick_clock.global_clock
        sems = wait_clock.sems.allocated()
        nc.sync.drain()
        for proc, tick in enumerate(list(clock)):
            if tick <= 0:
                continue
            handle = sems.get(proc)
            if handle is None:
                continue
            mult = 16 if handle.name.startswith("DMA") else 1
            nc.sync.wait_ge(handle, tick * mult)
        nc.all_engine_barrier()
        assert self.sems is not None
        popped = nc._tile_sem_poison_stack.pop()
        assert popped is self._sem_poison
        nc.clear_and_free_semaphores(list(self.sems.allocated().values()))
        nc.all_engine_barrier()

    TileContext._drain_and_barrier = _split_drain_and_barrier


NT = TOK // 512           # 16 token tiles of 512 for the projection
KT = 8                    # 1024 / 128 contraction tiles
QT = N // 512             # 4 q tiles per (b, h)
KTOK = N // 128           # 16 k-token tiles per (b, h)
VROW = 2 * (HD + 1)       # 130: [vA | 1 | vB | 1] per token tile


def build_graph(debug_dumps=False):
    nc = bacc.Bacc("TRN2", target_bir_lowering=False, debug=False)
    xt = nc.declare_dram_parameter("xt", [DIM, TOK], BF16, isOutput=False)
    wqk = nc.declare_dram_parameter("wqk", [DIM, 2 * HPC * HD], BF16, isOutput=False)
    wv = nc.declare_dram_parameter("wv", [DIM, HPC * HD], BF16, isOutput=False)
    bqk = nc.declare_dram_parameter("bqk", [2 * HPC * HD, 1], F32, isOutput=False)
    bv = nc.declare_dram_parameter("bv", [1, HPC * HD], BF16, isOutput=False)
    out = nc.declare_dram_parameter("out", [HPC, B, HD, N], F32, isOutput=True)
    F32R = mybir.dt.float32r
    NTB = N // 512            # 4 proj token-tiles per batch
    KTOK_B = N // 128         # 16 k-token tiles per batch

    with TileContext(nc) as tc:
        with (
            tc.tile_pool(name="const", bufs=1) as constp,
            tc.tile_pool(name="qk", bufs=1) as qkp,
            tc.tile_pool(name="xin", bufs=2) as xinp,
            tc.tile_pool(name="exps", bufs=28) as expp,
            tc.tile_pool(name="outs", bufs=4) as outp,
            tc.tile_pool(name="rcs", bufs=4) as rcp,
        ):
            # ---- constants ----
            wqk_s = constp.tile([128, KT * 256], BF16)
            for kt in range(KT):
                nc.sync.dma_start(out=wqk_s[:, kt * 256:(kt + 1) * 256],
                                  in_=wqk[kt * 128:(kt + 1) * 128, :])
            wv_s = constp.tile([128, KT * 128], BF16)
            for kt in range(KT):
                nc.sync.dma_start(out=wv_s[:, kt * 128:(kt + 1) * 128],
                                  in_=wv[kt * 128:(kt + 1) * 128, :])
            bqk_s = constp.tile([128, 2], F32)
            for mt in range(2):
                nc.sync.dma_start(out=bqk_s[:, mt:mt + 1],
                                  in_=bqk[mt * 128:(mt + 1) * 128, :])
            bv_s = constp.tile([1, 128], BF16)
            nc.sync.dma_start(out=bv_s[:, :], in_=bv[:, :])
            ones_s = constp.tile([1, 128], BF16)
            nc.vector.memset(ones_s[:, :], 1.0)
            onesb_s = constp.tile([65, 64], F32)       # bcast lhsT row 64
            nc.vector.memset(onesb_s[:, :], 1.0)

            # per-batch activation tensors (lets attention on batch b start
            # as soon as batch b's projection tiles land)
            q_sb = [qkp.tile([128, N], BF16, name=f"q_sb{_b}") for _b in range(B)]
            k_sb = [qkp.tile([128, N], BF16, name=f"k_sb{_b}") for _b in range(B)]
            v_sb = [qkp.tile([128, KTOK_B * VROW], BF16, name=f"v_sb{_b}") for _b in range(B)]
            for _b in range(B):
                nc.vector.memset(v_sb[_b][:, :], 1.0)

            # ---- phase B: QKV projection ----
            with (
                tc.tile_pool(name="qkps", bufs=2, space="PSUM") as qkps,
                tc.tile_pool(name="vps", bufs=2, space="PSUM") as vps,
            ):
                for nt in range(NT):
                    bb, ntb = nt // NTB, nt % NTB
                    xnt = xinp.tile([128, KT * 512], BF16)
                    for kt in range(KT):
                        nc.sync.dma_start(
                            out=xnt[:, kt * 512:(kt + 1) * 512],
                            in_=xt[kt * 128:(kt + 1) * 128, nt * 512:(nt + 1) * 512])
                    for mt in range(2):
                        ps = qkps.tile([128, 512], F32)
                        for kt in range(KT):
                            nc.tensor.matmul(
                                ps[:, :],
                                lhsT=wqk_s[:, kt * 256 + mt * 128: kt * 256 + (mt + 1) * 128],
                                rhs=xnt[:, kt * 512:(kt + 1) * 512],
                                start=(kt == 0), stop=(kt == KT - 1))
                        dst = q_sb[bb] if mt == 0 else k_sb[bb]
                        nc.vector.tensor_scalar_add(
                            dst[:, ntb * 512:(ntb + 1) * 512], ps[:, :],
                            bqk_s[:, mt:mt + 1])
                    for sub in range(4):
                        ttb = ntb * 4 + sub
                        vp = vps.tile([128, 128], F32)
                        for kt in range(KT):
                            nc.tensor.matmul(
                                vp[:, :],
                                lhsT=xnt[:, kt * 512 + sub * 128: kt * 512 + (sub + 1) * 128],
                                rhs=wv_s[:, kt * 128:(kt + 1) * 128],
                                start=(kt == 0), stop=False)
                        nc.tensor.matmul(vp[:, :], lhsT=ones_s[:, :], rhs=bv_s[:, :],
                                         start=False, stop=True)
                        nc.vector.tensor_copy(
                            v_sb[bb][:, ttb * VROW: ttb * VROW + HD], vp[:, 0:HD])
                        nc.vector.tensor_copy(
                            v_sb[bb][:, ttb * VROW + HD + 1: ttb * VROW + 2 * HD + 1],
                            vp[:, HD:2 * HD])

            if debug_dumps:
                dq = nc.declare_dram_parameter("dq", [128, N], F32, isOutput=True)
                dqs = outp.tile([128, N], F32, name="dqs", tag="dbg", bufs=1)
                nc.vector.tensor_copy(dqs[:, :], q_sb[0][:, :])
                nc.sync.dma_start(out=dq[:, :], in_=dqs[:, :])

            # ---- phase C: attention ----
            # wave = (b, qt).  Per k-tile: 2 row-tiled score MMs (heads at
            # rows 0-63 / 64-127) into a [128,1024] chunk, exp -> SBUF bf16;
            # then col-tiled av MMs (head A -> psum rows 0-63, head B ->
            # 64-127) and col-paired M=1 denominator MMs; normalize both
            # heads in single [128,512] DVE ops.
            with (
                tc.tile_pool(name="sps", bufs=3, space="PSUM") as sps,
                tc.tile_pool(name="avps", bufs=1, space="PSUM") as avps,
            ):
                for b in range(B):
                    for qt in range(QT):
                        qcol = qt * 512
                        echunks = []
                        for kt in range(KTOK_B):
                            kcol = kt * 128
                            s2 = sps.tile([128, 1024], F32, name="s2", tag="s2")
                            for h in range(2):
                                nc.tensor.matmul(
                                    s2[:, h * 512:(h + 1) * 512],
                                    lhsT=k_sb[b][h * 64:(h + 1) * 64, kcol:kcol + 128],
                                    rhs=q_sb[b][h * 64:(h + 1) * 64, qcol:qcol + 512],
                                    start=True, stop=True,
                                    tile_position=(h * 64, 0))
                            e2 = expp.tile([128, 1024], BF16, name="e2", tag="e2")
                            nc.scalar.activation(e2[:, :], s2[:, :], AF.Exp)
                            echunks.append(e2)
                        av = [avps.tile([65, 512], F32, name=f"av{_h}", tag=f"av{_h}", bufs=1)
                              for _h in range(2)]
                        for h in range(2):
                            for kt in range(KTOK_B):
                                nc.tensor.matmul(
                                    av[h][:, :],
                                    lhsT=v_sb[b][:, kt * VROW + h * (HD + 1): kt * VROW + (h + 1) * (HD + 1)],
                                    rhs=echunks[kt][:, h * 512:(h + 1) * 512],
                                    start=(kt == 0), stop=(kt == KTOK_B - 1),
                                    skip_group_check=True)
                        for h in range(2):
                            dn = rcp.tile([65, 512], F32, name="dn", tag="dn")
                            nc.vector.tensor_copy(dn[64:65, :], av[h][64:65, :])
                            bc = sps.tile([64, 512], F32, name="bc", tag="s2")
                            nc.tensor.matmul(bc[:, :], lhsT=onesb_s[64:65, 0:64],
                                             rhs=dn[64:65, :], start=True, stop=True)
                            bcs = rcp.tile([64, 512], F32, name="bcs", tag="bcs")
                            nc.vector.tensor_copy(bcs[:, :], bc[:, :])
                            nc.vector.reciprocal_approx_fast(bcs[:, :], bcs[:, :])
                            ot = outp.tile([64, 512], F32)
                            nc.vector.tensor_mul(ot[:, :], av[h][0:64, :], bcs[:, :])
                            nc.sync.dma_start(
                                out=out[h, b, :, qt * 512:(qt + 1) * 512],
                                in_=ot[:, :])
    nc.compile()
    return nc


_GRAPH = None


def _get_graph():
    global _GRAPH
    if _GRAPH is None:
        _GRAPH = build_graph()
    return _GRAPH


def _make_in_maps(x, w_qkv, b_qkv):
    bf = ml_dtypes.bfloat16
    xt = np.ascontiguousarray(x.reshape(TOK, DIM).T).astype(bf)
    in_maps = []
    for c in range(NCORES):
        hA, hB = HPC * c, HPC * c + 1
        rq = [w_qkv[h * HD:(h + 1) * HD] * SCALE for h in (hA, hB)]
        rk = [w_qkv[DIM + h * HD: DIM + (h + 1) * HD] for h in (hA, hB)]
        rv = [w_qkv[2 * DIM + h * HD: 2 * DIM + (h + 1) * HD] for h in (hA, hB)]
        wqk_c = np.ascontiguousarray(np.concatenate(rq + rk, axis=0).T).astype(bf)
        wv_c = np.ascontiguousarray(np.concatenate(rv, axis=0).T).astype(bf)
        bq = [b_qkv[h * HD:(h + 1) * HD] * SCALE for h in (hA, hB)]
        bk = [b_qkv[DIM + h * HD: DIM + (h + 1) * HD] for h in (hA, hB)]
        bvc = [b_qkv[2 * DIM + h * HD: 2 * DIM + (h + 1) * HD] for h in (hA, hB)]
        bqk_c = np.concatenate(bq + bk).astype(np.float32).reshape(-1, 1)
        bv_c = np.concatenate(bvc).astype(bf).reshape(1, -1)
        in_maps.append({"xt": xt, "wqk": wqk_c, "wv": wv_c,
                        "bqk": np.ascontiguousarray(bqk_c),
                        "bv": np.ascontiguousarray(bv_c)})
    return in_maps


def _run(x, w_qkv, b_qkv, trace=False, tmpdir=None):
    nc = _get_graph()
    in_maps = _make_in_maps(np.asarray(x, dtype=np.float32),
                            np.asarray(w_qkv, dtype=np.float32),
                            np.asarray(b_qkv, dtype=np.float32))
    res = run_bass_kernel_spmd(nc, in_maps, core_ids=list(range(NCORES)),
                               trace=trace, tmpdir=tmpdir)
    full = np.empty((B, N, DIM), dtype=np.float32)
    for c in range(NCORES):
        oc = res.results[c]["out"]          # [HPC, B, HD, N]
        # out[b, q, (HPC*c+hh)*HD + d] = oc[hh, b, d, q]
        full[:, :, c * HPC * HD:(c + 1) * HPC * HD] = \
            oc.transpose(1, 3, 0, 2).reshape(B, N, HPC * HD)
    return full, res


def kernel(x, w_qkv, b_qkv):
    full, _ = _run(x, w_qkv, b_qkv, trace=False)
    return full


# revision 18
# speedup vs baseline: 1.1875x; 1.1040x over previous
"""Multi-head attention (B=4, N=2048, DIM=1024, H=16, HD=64) on 8 TRN2 cores.

Sharding: tensor-parallel over heads — 2 heads per core. The reference omits
the output projection, so each core's output is a disjoint 128-column slice of
the final [B, N, 1024]; no collectives are needed.

Per-core device kernel (bf16 compute, fp32 PSUM accumulation):
  - QKV projection from a single pass over x^T: q^T,k^T produced transposed
    [outch, tokens] (weights stationary), v produced natural [tokens, outch]
    (x tiles stationary) with bias folded in as a K=1 accumulation.
  - scores^T = k^T.T @ q^T per (batch, head): K=64 contraction; head A lives
    on partitions 0-63 and head B on 64-127, so the two heads' matmuls
    row-tile the PE array and run concurrently.
  - exp on ScalarE over two-bank [128, 1024] PSUM tiles -> bf16 SBUF.
  - out^T = [v | 1]^T @ expT accumulated over k tiles; row 64 is the softmax
    denominator. Normalization: DVE fast reciprocal of row 64, broadcast to
    64 partitions via a K=1 PE outer product, DVE multiply, DMA out.
"""

import numpy as np
import ml_dtypes

import concourse.bass as bass
import concourse.bacc as bacc
import concourse.mybir as mybir
from concourse.bass_utils import run_bass_kernel_spmd
from concourse.tile import TileContext

B, N, DIM, H = 4, 2048, 1024, 16
HD = DIM // H
SCALE = 1.0 / np.sqrt(HD)
TOK = B * N               # 8192 tokens
NCORES = 8
HPC = H // NCORES         # heads per core = 2

BF16 = mybir.dt.bfloat16
F32 = mybir.dt.float32
AF = mybir.ActivationFunctionType


def _patch_tile_drain():
    """walrus here rejects Drain instructions carrying >2 sem waits; emit the
    waits as standalone single-wait instructions instead."""
    def _split_drain_and_barrier(self, tick_clock, wait_clock):
        nc = self.nc
        clock = tick_clock.global_clock
        sems = wait_clock.sems.allocated()
        nc.sync.drain()
        for proc, tick in enumerate(list(clock)):
            if tick <= 0:
                continue
            handle = sems.get(proc)
            if handle is None:
                continue
            mult = 16 if handle.name.startswith("DMA") else 1
            nc.sync.wait_ge(handle, tick * mult)
        nc.all_engine_barrier()
        assert self.sems is not None
        popped = nc._tile_sem_poison_stack.pop()
        assert popped is self._sem_poison
        nc.clear_and_free_semaphores(list(self.sems.allocated().values()))
        nc.all_engine_barrier()

    TileContext._drain_and_barrier = _split_drain_and_barrier


NT = TOK // 512           # 16 token tiles of 512 for the projection
KT = 8                    # 1024 / 128 contraction tiles
QT = N // 512             # 4 q tiles per (b, h)
KTOK = N // 128           # 16 k-token tiles per (b, h)
VROW = 2 * (HD + 1)       # 130: [vA | 1 | vB | 1] per token tile


def build_graph(debug_dumps=False):
    nc = bacc.Bacc("TRN2", target_bir_lowering=False, debug=False)
    xt = nc.declare_dram_parameter("xt", [DIM, TOK], BF16, isOutput=False)
    wqk = nc.declare_dram_parameter("wqk", [DIM, 2 * HPC * HD], BF16, isOutput=False)
    wv = nc.declare_dram_parameter("wv", [DIM, HPC * HD], BF16, isOutput=False)
    bqk = nc.declare_dram_parameter("bqk", [2 * HPC * HD, 1], F32, isOutput=False)
    bv = nc.declare_dram_parameter("bv", [1, HPC * HD], BF16, isOutput=False)
    out = nc.declare_dram_parameter("out", [HPC, B, HD, N], F32, isOutput=True)
    F32R = mybir.dt.float32r
    NTB = N // 512            # 4 proj token-tiles per batch
    KTOK_B = N // 128         # 16 k-token tiles per batch

    with TileContext(nc) as tc:
        with (
            tc.tile_pool(name="const", bufs=1) as constp,
            tc.tile_pool(name="qk", bufs=1) as qkp,
            tc.tile_pool(name="xin", bufs=2) as xinp,
            tc.tile_pool(name="exps", bufs=28) as expp,
            tc.tile_pool(name="outs", bufs=4) as outp,
            tc.tile_pool(name="rcs", bufs=4) as rcp,
        ):
            # ---- constants ----
            wqk_s = constp.tile([128, KT * 256], BF16)
            for kt in range(KT):
                nc.sync.dma_start(out=wqk_s[:, kt * 256:(kt + 1) * 256],
                                  in_=wqk[kt * 128:(kt + 1) * 128, :])
            wv_s = constp.tile([128, KT * 128], BF16)
            for kt in range(KT):
                nc.sync.dma_start(out=wv_s[:, kt * 128:(kt + 1) * 128],
                                  in_=wv[kt * 128:(kt + 1) * 128, :])
            bqk_s = constp.tile([128, 2], F32)
            for mt in range(2):
                nc.sync.dma_start(out=bqk_s[:, mt:mt + 1],
                                  in_=bqk[mt * 128:(mt + 1) * 128, :])
            bv_s = constp.tile([1, 128], BF16)
            nc.sync.dma_start(out=bv_s[:, :], in_=bv[:, :])
            ones_s = constp.tile([1, 128], BF16)
            nc.vector.memset(ones_s[:, :], 1.0)

            # per-batch activation tensors (lets attention on batch b start
            # as soon as batch b's projection tiles land)
            q_sb = [qkp.tile([128, N], BF16, name=f"q_sb{_b}") for _b in range(B)]
            k_sb = [qkp.tile([128, N], BF16, name=f"k_sb{_b}") for _b in range(B)]
            v_sb = [qkp.tile([128, KTOK_B * VROW], BF16, name=f"v_sb{_b}") for _b in range(B)]
            for _b in range(B):
                nc.vector.memset(v_sb[_b][:, :], 1.0)

            # ---- phase B: QKV projection ----
            with (
                tc.tile_pool(name="qkps", bufs=2, space="PSUM") as qkps,
                tc.tile_pool(name="vps", bufs=2, space="PSUM") as vps,
            ):
                for nt in range(NT):
                    bb, ntb = nt // NTB, nt % NTB
                    xnt = xinp.tile([128, KT * 512], BF16)
                    for kt in range(KT):
                        nc.sync.dma_start(
                            out=xnt[:, kt * 512:(kt + 1) * 512],
                            in_=xt[kt * 128:(kt + 1) * 128, nt * 512:(nt + 1) * 512])
                    for mt in range(2):
                        ps = qkps.tile([128, 512], F32)
                        for kt in range(KT):
                            nc.tensor.matmul(
                                ps[:, :],
                                lhsT=wqk_s[:, kt * 256 + mt * 128: kt * 256 + (mt + 1) * 128],
                                rhs=xnt[:, kt * 512:(kt + 1) * 512],
                                start=(kt == 0), stop=(kt == KT - 1))
                        dst = q_sb[bb] if mt == 0 else k_sb[bb]
                        nc.vector.tensor_scalar_add(
                            dst[:, ntb * 512:(ntb + 1) * 512], ps[:, :],
                            bqk_s[:, mt:mt + 1])
                    for sub in range(4):
                        ttb = ntb * 4 + sub
                        vp = vps.tile([128, 128], F32)
                        for kt in range(KT):
                            nc.tensor.matmul(
                                vp[:, :],
                                lhsT=xnt[:, kt * 512 + sub * 128: kt * 512 + (sub + 1) * 128],
                                rhs=wv_s[:, kt * 128:(kt + 1) * 128],
                                start=(kt == 0), stop=False)
                        nc.tensor.matmul(vp[:, :], lhsT=ones_s[:, :], rhs=bv_s[:, :],
                                         start=False, stop=True)
                        nc.vector.tensor_copy(
                            v_sb[bb][:, ttb * VROW + 1: ttb * VROW + 1 + HD],
                            vp[:, 0:HD])
                        nc.vector.tensor_copy(
                            v_sb[bb][:, ttb * VROW + HD + 2: ttb * VROW + 2 * HD + 2],
                            vp[:, HD:2 * HD])

            if debug_dumps:
                dq = nc.declare_dram_parameter("dq", [128, N], F32, isOutput=True)
                dqs = outp.tile([128, N], F32, name="dqs", tag="dbg", bufs=1)
                nc.vector.tensor_copy(dqs[:, :], q_sb[0][:, :])
                nc.sync.dma_start(out=dq[:, :], in_=dqs[:, :])

            # ---- phase C: attention ----
            # wave = (b, qt).  Per k-tile: 2 row-tiled score MMs (heads at
            # rows 0-63 / 64-127) into a [128,1024] chunk, exp -> SBUF bf16;
            # then col-tiled av MMs (head A -> psum rows 0-63, head B ->
            # 64-127) and col-paired M=1 denominator MMs; normalize both
            # heads in single [128,512] DVE ops.
            with (
                tc.tile_pool(name="sps", bufs=3, space="PSUM") as sps,
                tc.tile_pool(name="avps", bufs=1, space="PSUM") as avps,
            ):
                for b in range(B):
                    for qt in range(QT):
                        qcol = qt * 512
                        echunks = []
                        for kt in range(KTOK_B):
                            kcol = kt * 128
                            s2 = sps.tile([128, 1024], F32, name="s2", tag="s2")
                            for h in range(2):
                                nc.tensor.matmul(
                                    s2[:, h * 512:(h + 1) * 512],
                                    lhsT=k_sb[b][h * 64:(h + 1) * 64, kcol:kcol + 128],
                                    rhs=q_sb[b][h * 64:(h + 1) * 64, qcol:qcol + 512],
                                    start=True, stop=True,
                                    tile_position=(h * 64, 0))
                            e2 = expp.tile([128, 1024], BF16, name="e2", tag="e2")
                            nc.scalar.activation(e2[:, :], s2[:, :], AF.Exp)
                            echunks.append(e2)
                        av = [avps.tile([65, 512], F32, name=f"av{_h}", tag=f"av{_h}", bufs=1)
                              for _h in range(2)]
                        for h in range(2):
                            for kt in range(KTOK_B):
                                nc.tensor.matmul(
                                    av[h][:, :],
                                    lhsT=v_sb[b][:, kt * VROW + h * (HD + 1): kt * VROW + (h + 1) * (HD + 1)],
                                    rhs=echunks[kt][:, h * 512:(h + 1) * 512],
                                    start=(kt == 0), stop=(kt == KTOK_B - 1),
                                    skip_group_check=True)
                        for h in range(2):
                            dn = rcp.tile([1, 512], F32, name="dn", tag="dn")
                            nc.vector.tensor_copy(dn[0:1, :], av[h][0:1, :])
                            rc = rcp.tile([1, 512], F32, name="rc", tag="rc")
                            nc.vector.reciprocal_approx_fast(rc[0:1, :], dn[0:1, :])
                            bcs = rcp.tile([65, 512], F32, name="bcs", tag="bcs")
                            nc.gpsimd.partition_broadcast(bcs[:, :], rc[0:1, :])
                            ot = outp.tile([65, 512], F32)
                            nc.vector.tensor_mul(ot[0:65, :], av[h][0:65, :],
                                                 bcs[0:65, :])
                            nc.sync.dma_start(
                                out=out[h, b, :, qt * 512:(qt + 1) * 512],
                                in_=ot[1:65, :])
    nc.compile()
    return nc


_GRAPH = None


def _get_graph():
    global _GRAPH
    if _GRAPH is None:
        _GRAPH = build_graph()
    return _GRAPH


def _make_in_maps(x, w_qkv, b_qkv):
    bf = ml_dtypes.bfloat16
    xt = np.ascontiguousarray(x.reshape(TOK, DIM).T).astype(bf)
    in_maps = []
    for c in range(NCORES):
        hA, hB = HPC * c, HPC * c + 1
        rq = [w_qkv[h * HD:(h + 1) * HD] * SCALE for h in (hA, hB)]
        rk = [w_qkv[DIM + h * HD: DIM + (h + 1) * HD] for h in (hA, hB)]
        rv = [w_qkv[2 * DIM + h * HD: 2 * DIM + (h + 1) * HD] for h in (hA, hB)]
        wqk_c = np.ascontiguousarray(np.concatenate(rq + rk, axis=0).T).astype(bf)
        wv_c = np.ascontiguousarray(np.concatenate(rv, axis=0).T).astype(bf)
        bq = [b_qkv[h * HD:(h + 1) * HD] * SCALE for h in (hA, hB)]
        bk = [b_qkv[DIM + h * HD: DIM + (h + 1) * HD] for h in (hA, hB)]
        bvc = [b_qkv[2 * DIM + h * HD: 2 * DIM + (h + 1) * HD] for h in (hA, hB)]
        bqk_c = np.concatenate(bq + bk).astype(np.float32).reshape(-1, 1)
        bv_c = np.concatenate(bvc).astype(bf).reshape(1, -1)
        in_maps.append({"xt": xt, "wqk": wqk_c, "wv": wv_c,
                        "bqk": np.ascontiguousarray(bqk_c),
                        "bv": np.ascontiguousarray(bv_c)})
    return in_maps


def _run(x, w_qkv, b_qkv, trace=False, tmpdir=None):
    nc = _get_graph()
    in_maps = _make_in_maps(np.asarray(x, dtype=np.float32),
                            np.asarray(w_qkv, dtype=np.float32),
                            np.asarray(b_qkv, dtype=np.float32))
    res = run_bass_kernel_spmd(nc, in_maps, core_ids=list(range(NCORES)),
                               trace=trace, tmpdir=tmpdir)
    full = np.empty((B, N, DIM), dtype=np.float32)
    for c in range(NCORES):
        oc = res.results[c]["out"]          # [HPC, B, HD, N]
        # out[b, q, (HPC*c+hh)*HD + d] = oc[hh, b, d, q]
        full[:, :, c * HPC * HD:(c + 1) * HPC * HD] = \
            oc.transpose(1, 3, 0, 2).reshape(B, N, HPC * HD)
    return full, res


def kernel(x, w_qkv, b_qkv):
    full, _ = _run(x, w_qkv, b_qkv, trace=False)
    return full
